# revision 6
# baseline (speedup 1.0000x reference)
"""Trainium2 Bass kernel for nn_Attention_54580444397738 (gnn_message_passing) v5.

Math per batch b (B=8, N=128, H=256, C=16):
  proj         = local @ W_apair                                     [N, H]
  pre[i,j,:]   = proj[i,:] + proj[j,:] + binary[i,j,:] @ W_binary
                 + b_apair + b_binary                                [N, N, H]
  score[i,j]   = sigmoid(relu(pre[i,j,:]) . W_att + b_att)           [N, N]
  glob         = score @ local                                       [N, H]
  local_pair [i,j,:] = local[i,:] + local[j,:]                       (output 1)
  global_pair[i,j,:] = glob[i,:]  + glob[j,:]                        (output 2)

v5 (after HW A/B probes of v4):
  - Flat rows ([1, N*H] partition-0 copies feeding partition_broadcast)
    go back to direct SBUF->SBUF gathers: the v4 DRAM-bounce load was
    ~30+us on HW (HBM -> single-partition is port-serialized), while the
    SBUF->SBUF fold is cheap on HW even though the rust cost model
    charges it 25us.  Gathers ride the scalar ring; the SP ring carries
    binary + the 16 x 1MB output writes only.
  - All output-stage adds on DVE ('D'); gpsimd tensor_tensor measured
    ~5.8us/stage on HW (vs 2.2 DVE).
  - Attention instruction-count diet (HW runs ~2x the modeled time,
    consistent with per-instruction overhead + PE clock ramping):
      * 4-j PSUM tiles (two banks) -> 32 relus of FD=1024 (was 64x512)
      * identB pass merged across the j-pair (broadcast rhs, N=512)
      * binT + bias matmul merged across the j-pair via a host-built
        block-diagonal [128, 512] weight tile (K=49 spans both c-blocks)
    -> 6 matmuls + 1 activation per 4 j's (was 10 + 2).
"""

import numpy as np

B, N, H, BIN = 8, 128, 256, 16
NCORES = 8
CPAD = 32        # c dim padded 16 -> 32 so transposed blocks land 32-aligned
IG = 4           # i's per binary-transpose group (4 * 32 = 128)
JS = 16          # j's per output stage tile
NSTAGE = N // JS

# output-stage variants: D = gpsimd bcast + DVE add, P = gpsimd bcast +
# gpsimd add, E = PE compose + ACT cast.
LP_PAT = "DDDDDDDD"
GP_PAT = "DDDDDDDD"
# output-write ring per stage: S = nc.sync (SP HWDGE), A = nc.scalar.
LP_RING = "SSSSSSSS"
GP_RING = "SSSSSSSS"
SKIP_ATTN = False     # probe knob: drop attention/score work (wrong gp)
SKIP_OUT = False      # probe knob: drop output stages (no lp/gp writes)
SKIP_ACT = False      # probe knob: attention matmuls only (no relu/reduce)

_cache = {}


def _body(tc, io, P, reps=1):
    import concourse.bass as bass
    import concourse.mybir as mybir
    from concourse.masks import make_identity
    from contextlib import ExitStack, nullcontext

    nc = tc.nc
    ts = bass.ts
    f32 = mybir.dt.float32
    f32r = mybir.dt.float32r
    bf16 = mybir.dt.bfloat16
    Relu = mybir.ActivationFunctionType.Relu
    Sigmoid = mybir.ActivationFunctionType.Sigmoid
    AX = mybir.AxisListType.X
    ADD = mybir.AluOpType.add

    local_d, binary_d, wap_d, wxbd_d, batt_d, lp_d, gp_d = io

    any_e = ("E" in LP_PAT or "E" in GP_PAT) and not SKIP_OUT

    ctx = ExitStack()
    with ctx:
        persist = ctx.enter_context(tc.tile_pool(name="persist", bufs=1))
        a2p = ctx.enter_context(tc.tile_pool(name="a2p", bufs=3))
        stagep = ctx.enter_context(tc.tile_pool(name="stagep", bufs=4))
        bcastp = ctx.enter_context(tc.tile_pool(name="bcastp", bufs=3))
        flatp = ctx.enter_context(tc.tile_pool(name="flatp", bufs=1))
        prep = ctx.enter_context(tc.tile_pool(name="prep", bufs=3, space="PSUM"))
        tpp = ctx.enter_context(tc.tile_pool(name="tpp", bufs=2, space="PSUM"))
        pep = (ctx.enter_context(tc.tile_pool(name="pep", bufs=1, space="PSUM"))
               if any_e else None)

        # timing builds wrap the whole body in a device-side loop
        loop = tc.For_i(0, reps, 1) if reps > 1 else nullcontext()
        ctx.enter_context(loop)

        # ---------------- persistent setup ----------------
        localSb = persist.tile([N, H], f32, tag="localSb")
        nc.scalar.dma_start(out=localSb, in_=local_d)
        xbL = persist.tile([N, H], bf16, tag="xbL")
        nc.vector.tensor_copy(out=xbL, in_=localSb)
        flL = flatp.tile([1, N * H], bf16, tag="flat")
        nc.scalar.dma_start(out=flL, in_=xbL)

        # binary loads CONTIGUOUSLY as [i, (j,c)] on the (otherwise idle
        # before writes) SP ring; pad c 16->32, ones lane at c=16, ZEROS at
        # c=17..31 (the merged K=49 matmul streams those rows against the
        # zero rows of wxbd -- NaN garbage there would poison the product).
        binRaw = persist.tile([128, N * BIN], f32, tag="binRaw")
        nc.sync.dma_start(out=binRaw, in_=binary_d.rearrange("i j c -> i (j c)"))
        binp = persist.tile([128, N * CPAD], f32, tag="binp")
        binp3 = binp.rearrange("p (j c) -> p j c", c=CPAD)
        nc.vector.memset(binp3[:, :, 16:CPAD], 0.0)
        nc.vector.memset(binp3[:, :, 16:17], 1.0)
        nc.scalar.copy(
            out=binp3[:, :, 0:BIN],
            in_=binRaw.rearrange("p (j c) -> p j c", c=BIN))

        identity = persist.tile([128, 128], f32, tag="identity")
        make_identity(nc, identity)
        identB = persist.tile([128, 128], bf16, tag="identB")
        nc.scalar.copy(out=identB, in_=identity)
        onesT = persist.tile([128, 128], bf16, tag="onesT")
        nc.vector.memset(onesT, 1.0)
        onesB = persist.tile([1, 128], bf16, tag="onesB")
        nc.vector.memset(onesB, 1.0)

        # f32 loads, converted to f32r/bf16 by compute-engine copies
        wapF = persist.tile([128, 2 * H], f32, tag="wapF")
        nc.scalar.dma_start(out=wapF[:, 0:H], in_=wap_d[0:128])
        nc.scalar.dma_start(out=wapF[:, H : 2 * H], in_=wap_d[128:256])
        wapR = persist.tile([128, 2 * H], f32r, tag="wapR")
        nc.scalar.copy(out=wapR, in_=wapF)

        wxbdF = persist.tile([128, 2 * H], f32, tag="wxbdF")
        nc.scalar.dma_start(out=wxbdF, in_=wxbd_d)
        wxbdB = persist.tile([128, 2 * H], bf16, tag="wxbdB")
        nc.scalar.copy(out=wxbdB, in_=wxbdF)

        battRow = persist.tile([1, 1], f32, tag="battRow")
        nc.scalar.dma_start(out=battRow, in_=batt_d.unsqueeze(0))
        battCol = persist.tile([128, 1], f32, tag="battCol")
        nc.gpsimd.partition_broadcast(battCol, battRow)

        # localT = local^T (f32r), then projW = local @ W_apair' (f32r)
        localT = persist.tile([128, H], f32r, tag="localT")
        for hb in range(2):
            tp = tpp.tile([128, 128], f32, tag="tp")
            nc.tensor.transpose(tp, localSb[:, ts(hb, 128)], identity)
            nc.scalar.copy(out=localT[:, ts(hb, 128)], in_=tp)
        pp = prep.tile([128, 4 * H], f32, tag="pre")
        nc.tensor.matmul(pp[:, 0:H], lhsT=localT[:, 0:128], rhs=wapR[:, 0:H],
                         start=True, stop=False)
        nc.tensor.matmul(pp[:, 0:H], lhsT=localT[:, 128:256], rhs=wapR[:, H : 2 * H],
                         start=False, stop=True)
        projWb = persist.tile([128, H], bf16, tag="projWb")
        nc.scalar.copy(out=projWb, in_=pp[:, 0:H])

        # proj rows (bf16) flattened to partitions {0,32,64,96} for the
        # ones-pass rhs -- one SBUF->SBUF gather on the scalar ring
        projFlat = persist.tile([128, 32 * H], bf16, tag="projFlat")
        nc.scalar.dma_start(
            out=projFlat.rearrange("(a x) f -> a x f", x=32)[:, 0, :],
            in_=projWb)

        # ALL 32 binary transposes + casts run up front: the per-chunk
        # transpose->cast->matmul chain made the PE micro-idle every 4 j's,
        # which HAM-throttles the clock to 1.2 GHz; hoisting them gives the
        # attention matmul stream zero cross-engine waits (measured 2x).
        binTall = persist.tile([128, 32 * 128], bf16, tag="binTall")
        for g in range(32):
            tp = tpp.tile([128, 128], f32, tag="tp")
            nc.tensor.transpose(tp, binp[:, ts(g, 128)], identity)
            nc.scalar.copy(out=binTall[:, ts(g, 128)], in_=tp)

        sp = persist.tile([128, N], f32, tag="sp")
        sm = persist.tile([128, N], f32, tag="sm")
        logitsT = persist.tile([128, N], f32, tag="logitsT")

        # ---------------- helpers ----------------
        a2tiles = {}
        projB2 = projWb.unsqueeze(1).broadcast_to([128, 2, H])

        def attn_chunk(c):
            """pre/relu for j in [8c, 8c+8): two 4-j PSUM tiles (2 banks
            each).  Per 4j: 1 binary transpose + cast, then per j-pair one
            ones-row matmul (proj[j],proj[j+1] row bcast), one merged
            identB matmul (+proj[i,k] to both halves), one merged K=49
            binT x block-diag-W matmul (binary term + bias, both j's).
            One relu (FD=1024) per 4j.  The dot-reduces are emitted one
            stage later (attn_reduce) so DVE never stalls fresh tiles."""
            a2 = a2p.tile([128, 8 * H], bf16, tag="a2")
            a2tiles[c] = a2
            for jj in (0, 4):
                j0 = 8 * c + jj
                g = j0 // IG
                bt = binTall[:, ts(g, 128)]
                pre = prep.tile([128, 4 * H], f32, tag="pre")
                for h2 in range(2):
                    j = j0 + 2 * h2
                    q, r = divmod(j, 32)
                    sl = pre[:, 2 * h2 * H : (2 * h2 + 2) * H]
                    nc.tensor.matmul(
                        sl, lhsT=onesT[32 * q : 32 * q + 1, :],
                        rhs=projFlat[32 * q : 32 * q + 1, r * H : (r + 2) * H],
                        start=True, stop=False, tile_position=(32 * q, 0))
                    nc.tensor.matmul(sl, lhsT=identB, rhs=projB2,
                                     start=False, stop=False)
                    jl = j % IG  # 0 or 2
                    nc.tensor.matmul(
                        sl, lhsT=bt[32 * jl : 32 * jl + 49, :],
                        rhs=wxbdB[32 * jl : 32 * jl + 49, :],
                        start=False, stop=True, tile_position=(32 * jl, 0))
                if not SKIP_ACT:
                    nc.scalar.activation(out=a2[:, jj * H : (jj + 4) * H],
                                         in_=pre, func=Relu)

        def attn_reduce(c):
            a2 = a2tiles.pop(c)
            if SKIP_ACT:
                return
            a3 = a2.rearrange("p (g k) -> p g k", k=H)
            if P > 0:
                nc.vector.tensor_reduce(out=sp[:, ts(c, 8)], in_=a3[:, :, 0:P],
                                        axis=AX, op=ADD)
            if P < H:
                nc.vector.tensor_reduce(out=sm[:, ts(c, 8)], in_=a3[:, :, P:H],
                                        axis=AX, op=ADD)

        def load_flat(xb):
            fl = flatp.tile([1, N * H], bf16, tag="flat")
            nc.scalar.dma_start(out=fl, in_=xb)
            return fl

        def out_stage(s, v, ring, xb, flA, dram_out):
            fl = flA[0:1, s * JS * H : (s + 1) * JS * H]
            stage = stagep.tile([128, JS * H], bf16, tag="stage")
            st3 = stage.rearrange("p (j h) -> p j h", h=H)
            if v in "DP":
                bt = bcastp.tile([128, JS * H], bf16, tag="bt")
                # bitcast bf16 pairs to f32: partition_broadcast cost scales
                # with element count, so this halves the GPSIMD time
                nc.gpsimd.partition_broadcast(bt.bitcast(f32), fl.bitcast(f32))
                eng = nc.vector if v == "D" else nc.gpsimd
                eng.tensor_add(
                    out=st3, in0=xb.unsqueeze(1).broadcast_to([128, JS, H]),
                    in1=bt.rearrange("p (j h) -> p j h", h=H))
            else:  # 'E': all-PE compose in PSUM + ACT cast-copy
                xb2 = xb.unsqueeze(1).broadcast_to([128, 2, H])
                for p in range(8):
                    po = pep.tile([128, 2 * H], f32, tag="pe")
                    nc.tensor.matmul(po, lhsT=identB, rhs=xb2,
                                     start=True, stop=False)
                    nc.tensor.matmul(po, lhsT=onesB,
                                     rhs=fl[0:1, p * 2 * H : (p + 1) * 2 * H],
                                     start=False, stop=True)
                    nc.scalar.copy(out=stage[:, p * 2 * H : (p + 1) * 2 * H], in_=po)
            weng = nc.sync if ring == "S" else nc.scalar
            weng.dma_start(out=dram_out[:, ts(s, JS), :], in_=st3)

        # ---------------- phase 1: local_pair + attention ----------------
        for s in range(NSTAGE):
            if not SKIP_OUT:
                out_stage(s, LP_PAT[s], LP_RING[s], xbL, flL, lp_d)
            if not SKIP_ATTN:
                attn_chunk(2 * s)
                attn_chunk(2 * s + 1)
                if s > 0:
                    attn_reduce(2 * (s - 1))
                    attn_reduce(2 * (s - 1) + 1)
        if not SKIP_ATTN:
            attn_reduce(2 * (NSTAGE - 1))
            attn_reduce(2 * (NSTAGE - 1) + 1)

        # ---------------- scores -> glob ----------------
        xbG = persist.tile([N, H], bf16, tag="xbG")
        if SKIP_ATTN or SKIP_ACT:
            nc.vector.tensor_copy(out=xbG, in_=localSb)
        else:
            # logits/score are [i-part, j-free]; transpose for the glob MM
            score = persist.tile([128, N], f32, tag="score")
            if P == 0:
                nc.vector.tensor_scalar_mul(out=logitsT, in0=sm, scalar1=-1.0)
            elif P == H:
                nc.vector.tensor_copy(out=logitsT, in_=sp)
            else:
                nc.vector.tensor_sub(out=logitsT, in0=sp, in1=sm)
            nc.scalar.activation(out=score, in_=logitsT, func=Sigmoid,
                                 bias=battCol)
            tsc = tpp.tile([128, 128], f32, tag="tp")
            nc.tensor.transpose(tsc, score, identity)
            scoreT = persist.tile([128, N], f32, tag="scoreT")
            nc.scalar.copy(out=scoreT, in_=tsc)
            pg = prep.tile([128, 4 * H], f32, tag="pre")
            nc.tensor.matmul(pg[:, 0:H], lhsT=scoreT, rhs=localSb,
                             start=True, stop=True)
            nc.scalar.copy(out=xbG, in_=pg[:, 0:H])

        # ---------------- phase 2: global_pair ----------------
        if not SKIP_OUT:
            flG = load_flat(xbG)
            for s in range(NSTAGE):
                out_stage(s, GP_PAT[s], GP_RING[s], xbG, flG, gp_d)
        else:
            nc.sync.dma_start(out=gp_d[0:1, 0:1, :], in_=xbG[0:1, :])
            nc.sync.dma_start(out=lp_d[0:1, 0:1, :], in_=xbL[0:1, :])


def _build(P, reps=1):
    import concourse.bass as bass  # noqa: F401
    from concourse import bacc
    import concourse.mybir as mybir
    import concourse.tile as tile

    f32 = mybir.dt.float32
    bf16 = mybir.dt.bfloat16
    nc = bacc.Bacc(
        "TRN2",
        target_bir_lowering=False,
        debug=False,
        enable_asserts=False,
        num_devices=NCORES,
    )
    io = (
        nc.dram_tensor("local", [N, H], f32, kind="ExternalInput").ap(),
        nc.dram_tensor("binary", [N, N, BIN], f32, kind="ExternalInput").ap(),
        nc.dram_tensor("w_apair", [H, H], f32, kind="ExternalInput").ap(),
        nc.dram_tensor("wxbd", [128, 2 * H], f32, kind="ExternalInput").ap(),
        nc.dram_tensor("b_att", [1], f32, kind="ExternalInput").ap(),
        nc.dram_tensor("out_lp", [N, N, H], bf16, kind="ExternalOutput").ap(),
        nc.dram_tensor("out_gp", [N, N, H], bf16, kind="ExternalOutput").ap(),
    )
    with tile.TileContext(nc) as tc:
        _body(tc, io, P, reps=reps)
    nc.compile()
    return nc


def _prep_inputs(inputs):
    f = lambda x: np.ascontiguousarray(np.asarray(x), dtype=np.float32)
    w_att = f(inputs["W_att"]).reshape(-1)
    perm = np.argsort((w_att <= 0).astype(np.int32), kind="stable")
    P = int((w_att > 0).sum())
    a = np.abs(w_att[perm])
    wap = f(inputs["W_apair"])[:, perm] * a[None, :]
    wbin = f(inputs["W_binary"])[:, perm] * a[None, :]
    bias = (f(inputs["b_apair"]) + f(inputs["b_binary"]))[perm] * a
    # block-diagonal [128, 512]: rows 32m..32m+15 = W_binary, row 32m+16 =
    # bias, into columns (m%2)*256..+256; zeros elsewhere (matching the
    # zero-padded c-lanes 17..31 of the transposed binary tiles)
    wxbd = np.zeros((128, 2 * H), np.float32)
    for m in range(4):
        cols = slice((m % 2) * H, (m % 2) * H + H)
        wxbd[32 * m : 32 * m + 16, cols] = wbin
        wxbd[32 * m + 16, cols] = bias
    shared = {
        "w_apair": np.ascontiguousarray(wap),
        "wxbd": wxbd,
        "b_att": f(inputs["b_att"]),
    }
    local = f(inputs["local_feats"])
    binary = f(inputs["binary_feats"])
    in_maps = [
        {"local": local[c], "binary": binary[c], **shared} for c in range(NCORES)
    ]
    return P, in_maps


def _get_nc(P):
    if P not in _cache:
        _cache[P] = _build(P)
    return _cache[P]


def _run(inputs, trace=False):
    from concourse.bass_utils import run_bass_kernel_spmd

    P, in_maps = _prep_inputs(inputs)
    nc = _get_nc(P)
    res = run_bass_kernel_spmd(
        nc, in_maps, core_ids=list(range(NCORES)), trace=trace
    )
    lp = np.stack([np.asarray(r["out_lp"]).astype(np.float32)
                   for r in res.results])
    gp = np.stack([np.asarray(r["out_gp"]).astype(np.float32)
                   for r in res.results])
    return (lp, gp), res


def kernel(**inputs):
    out, _ = _run(inputs, trace=False)
    return out
